# revision 6
# baseline (speedup 1.0000x reference)
# Trainium2 Bass kernel for NeighborhoodAugmenter (retrieval_knn).
#
# reference semantics:
#   h_norm = latent / ||latent||            (rows)
#   sim    = h_norm @ h_norm.T;  diag -> -9e15
#   top3   = top_k(sim, 3) indices; pick rand_idx-th per row
#   out    = where(unif < 0.8, x, x[neighbor])
#
# Strategy (8 cores, batch-sharded, x replicated so the neighbor gather is
# local):
#   Per core (S=1024 rows): compute v = latent_shard @ hnT where hnT holds
#   column-normalized latent (so row ordering matches cosine sim and the
#   self-column is always rank-0).  top-8 via DVE max/max_index, neighbor =
#   idx8[:, 1 + rand_idx].  Indirect-DMA gather of neighbor rows of the full
#   x, then out = where(relu(0.8-unif)>0, x_shard, gathered).
import numpy as np

B, G, D = 8192, 20000, 64
N_CORES = 8
S = B // N_CORES          # rows per core
TP = 128                  # rows per row-tile
NT = S // TP              # row-tiles per core
W_G = 5000                # gather chunk width (20 KB per row descriptor)
W_M = 2500                # mix chunk width
MIX = 0.8

_PROG = None


def build_program(nc, b, g, s, w_g, w_m, x_name="xf"):
    import concourse.bass as bass
    import concourse.tile as tile
    from concourse import mybir
    from concourse.bass import ds, ts
    from concourse.masks import make_identity

    f32 = mybir.dt.float32
    i32 = mybir.dt.int32
    u32 = mybir.dt.uint32
    AX = mybir.AxisListType
    AF = mybir.ActivationFunctionType
    OP = mybir.AluOpType

    tp = 128
    nt = s // tp
    r = b // 128              # latent rows per partition in prep layout
    nj = b // 512             # 512-wide matmul column chunks

    xf = nc.dram_tensor(x_name, [b, g], f32, kind="ExternalInput").ap()
    lat = nc.dram_tensor("lat", [b, D], f32, kind="ExternalInput").ap()
    latTs = nc.dram_tensor("latTs", [D, s], f32, kind="ExternalInput").ap()
    xs = nc.dram_tensor("xs", [s, g], f32, kind="ExternalInput").ap()
    unif = nc.dram_tensor("unif", [s, g], f32, kind="ExternalInput").ap()
    rnd = nc.dram_tensor("rnd", [tp, nt], i32, kind="ExternalInput").ap()
    out = nc.dram_tensor("out", [s, g], f32, kind="ExternalOutput").ap()

    with tile.TileContext(nc) as tc:
        with tc.tile_pool(name="const", bufs=1) as cpool:
            ident = cpool.tile([128, 128], f32)
            make_identity(nc, ident[:])
            iota8 = cpool.tile([tp, 8], i32)
            nc.gpsimd.iota(iota8[:], pattern=[[1, 8]], base=-1, channel_multiplier=0)
            rnd_sb = cpool.tile([tp, nt], i32)
            nc.sync.dma_start(rnd_sb[:], rnd)
            latTs_sb = cpool.tile([D, s], f32)
            nc.sync.dma_start(latTs_sb[:], latTs)
            hnT = cpool.tile([D, b], f32)
            hnT_v = hnT[:].rearrange("d (p j) -> d p j", j=r)

            # ---- prep: column-normalized latent, transposed into hnT ----
            with (
                tc.tile_pool(name="prep", bufs=1) as ppool,
                tc.tile_pool(name="prep_ps", bufs=2, space="PSUM") as pps,
            ):
                hbig = ppool.tile([128, r, D], f32)
                nc.sync.dma_start(hbig[:], lat.rearrange("(p n) d -> p n d", p=128))
                sq = ppool.tile([128, r, D], f32)
                nc.vector.tensor_tensor(out=sq[:], in0=hbig[:], in1=hbig[:], op=OP.mult)
                ss = ppool.tile([128, r], f32)
                nc.vector.reduce_sum(out=ss[:], in_=sq[:], axis=AX.X)
                rec = ppool.tile([128, r], f32)
                nc.vector.reciprocal(out=rec[:], in_=ss[:])
                sc0 = ppool.tile([128, r], f32)
                nc.scalar.activation(out=sc0[:], in_=rec[:], func=AF.Sqrt)
                # one Newton step: sc = sc0 * (1.5 - 0.5*ss*sc0^2)
                t1 = ppool.tile([128, r], f32)
                nc.vector.tensor_tensor(out=t1[:], in0=ss[:], in1=sc0[:], op=OP.mult)
                nc.vector.tensor_tensor(out=t1[:], in0=t1[:], in1=sc0[:], op=OP.mult)
                nc.vector.tensor_scalar(
                    out=t1[:], in0=t1[:], scalar1=-0.5, scalar2=1.5,
                    op0=OP.mult, op1=OP.add,
                )
                sc = ppool.tile([128, r, 1], f32)
                nc.vector.tensor_tensor(out=sc[:, :, 0], in0=sc0[:], in1=t1[:], op=OP.mult)
                hn = ppool.tile([128, r, D], f32)
                nc.vector.tensor_tensor(
                    out=hn[:], in0=hbig[:], in1=sc[:].to_broadcast([128, r, D]), op=OP.mult
                )
                for j in range(r):
                    pt = pps.tile([D, 128], f32)
                    nc.tensor.transpose(out=pt[:], in_=hn[:, j, :], identity=ident[:])
                    nc.scalar.copy(out=hnT_v[:, :, j], in_=pt[:])

            def main_loop(nt):
                for t in range(nt):
                    sim = simpool.tile([tp, b], f32, tag="sim")
                    for j in range(nj):
                        ps = mmpool.tile([tp, 512], f32, tag="mm")
                        nc.tensor.matmul(
                            ps[:],
                            lhsT=latTs_sb[:, ts(t, tp)],
                            rhs=hnT[:, ts(j, 512)],
                            start=True,
                            stop=True,
                        )
                        nc.scalar.copy(out=sim[:, ts(j, 512)], in_=ps[:])
                    mx = spool.tile([tp, 8], f32, tag="mx")
                    nc.vector.max(out=mx[:], in_=sim[:])
                    mi = spool.tile([tp, 8], u32, tag="mi")
                    nc.vector.max_index(out=mi[:], in_max=mx[:], in_values=sim[:])
                    eq = spool.tile([tp, 8], i32, tag="eq")
                    nc.vector.tensor_tensor(
                        out=eq[:],
                        in0=iota8[:],
                        in1=rnd_sb[:, t : t + 1].to_broadcast([tp, 8]),
                        op=OP.is_equal,
                    )
                    pr = spool.tile([tp, 8], i32, tag="pr")
                    nc.vector.tensor_tensor(
                        out=pr[:], in0=eq[:], in1=mi[:].bitcast(i32), op=OP.mult
                    )
                    nbr = spool.tile([tp, 1], i32, tag="nbr")
                    with nc.allow_low_precision(reason="int32 index select, exact"):
                        nc.vector.reduce_sum(out=nbr[:], in_=pr[:], axis=AX.X)

                    for gi in range(g // w_g):
                        gt = gpool.tile([tp, w_g], tag="gt", dtype=f32)
                        nc.gpsimd.indirect_dma_start(
                            out=gt[:],
                            out_offset=None,
                            in_=xf,
                            in_offset=bass.IndirectOffsetOnAxis(ap=nbr[:, :1], axis=0),
                            element_offset=gi * w_g,
                        )
                        for h in range(w_g // w_m):
                            c0 = gi * w_g + h * w_m
                            xc = xpool.tile([tp, w_m], f32, tag="xc")
                            nc.sync.dma_start(xc[:], xs[ds(t * tp, tp), ds(c0, w_m)])
                            uc = upool.tile([tp, w_m], f32, tag="uc")
                            nc.sync.dma_start(uc[:], unif[ds(t * tp, tp), ds(c0, w_m)])
                            mk = mpool.tile([tp, w_m], mybir.dt.int8, tag="mk")
                            nc.vector.tensor_scalar(
                                out=mk[:], in0=uc[:], scalar1=MIX, scalar2=None,
                                op0=OP.is_lt,
                            )
                            nc.vector.copy_predicated(
                                out=gt[:, ds(h * w_m, w_m)], mask=mk[:], data=xc[:]
                            )
                        nc.sync.dma_start(out[ds(t * tp, tp), ds(gi * w_g, w_g)], gt[:])

            # ---- main loop over row-tiles ----
            with (
                tc.tile_pool(name="simp", bufs=1) as simpool,
                tc.tile_pool(name="gat", bufs=2) as gpool,
                tc.tile_pool(name="xch", bufs=3) as xpool,
                tc.tile_pool(name="uch", bufs=3) as upool,
                tc.tile_pool(name="msk", bufs=3) as mpool,
                tc.tile_pool(name="small", bufs=2) as spool,
                tc.tile_pool(name="mm", bufs=4, space="PSUM") as mmpool,
            ):
                main_loop(nt)
    return nc


def _get_prog():
    global _PROG
    if _PROG is None:
        from concourse import bacc

        nc = bacc.Bacc(
            "TRN2", target_bir_lowering=False, debug=False, num_devices=N_CORES
        )
        build_program(nc, B, G, S, W_G, W_M)
        nc.compile()
        _PROG = nc
    return _PROG


def make_in_maps(x, latent, rand_idx, unif):
    x = np.ascontiguousarray(np.asarray(x, dtype=np.float32))
    latent = np.ascontiguousarray(np.asarray(latent, dtype=np.float32))
    rand_idx = np.asarray(rand_idx, dtype=np.int32)
    unif = np.ascontiguousarray(np.asarray(unif, dtype=np.float32))
    in_maps = []
    for c in range(N_CORES):
        r0 = c * S
        in_maps.append(
            {
                "xf": x,
                "lat": latent,
                "latTs": np.ascontiguousarray(latent[r0 : r0 + S].T),
                "xs": x[r0 : r0 + S],
                "unif": unif[r0 : r0 + S],
                "rnd": np.ascontiguousarray(
                    rand_idx[r0 : r0 + S].reshape(NT, TP).T
                ),
            }
        )
    return in_maps


def kernel(x, latent, rand_idx, unif):
    from concourse.bass_utils import run_bass_kernel_spmd

    nc = _get_prog()
    in_maps = make_in_maps(x, latent, rand_idx, unif)
    res = run_bass_kernel_spmd(nc, in_maps, core_ids=list(range(N_CORES)))
    return np.concatenate([res.results[c]["out"] for c in range(N_CORES)], axis=0)



# revision 7
# speedup vs baseline: 1.0674x; 1.0674x over previous
# Trainium2 Bass kernel for NeighborhoodAugmenter (retrieval_knn).
#
# reference semantics:
#   h_norm = latent / ||latent||            (rows)
#   sim    = h_norm @ h_norm.T;  diag -> -9e15
#   top3   = top_k(sim, 3) indices; pick rand_idx-th per row
#   out    = where(unif < 0.8, x, x[neighbor])
#
# Strategy (8 cores, batch-sharded, x replicated so the neighbor gather is
# local):
#   Per core (S=1024 rows): v = latent_shard @ hnT with hnT the
#   column-normalized transposed latent (host-precomputed; row scaling does
#   not change per-row order, and the self column is always rank-0, which
#   replaces the diagonal mask).  Top-8 per row via DVE max/max_index,
#   neighbor = idx8[:, 1 + rand_idx].  Indirect-DMA gather of neighbor rows
#   of the full x, then out = where(unif < 0.8, x_shard, gathered).
import numpy as np

B, G, D = 8192, 20000, 64
N_CORES = 8
S = B // N_CORES          # rows per core
TP = 128                  # rows per row-tile
NT = S // TP              # row-tiles per core
W = 5000                  # chunk width (20 KB per row descriptor)
MIX = 0.8

_PROG = None


def build_program(nc, b, g, s, w):
    import concourse.bass as bass
    import concourse.tile as tile
    from concourse import mybir
    from concourse.bass import ds, ts

    f32 = mybir.dt.float32
    i32 = mybir.dt.int32
    u32 = mybir.dt.uint32
    i8 = mybir.dt.int8
    AX = mybir.AxisListType
    OP = mybir.AluOpType

    tp = 128
    nt = s // tp
    nj = b // 512             # 512-wide matmul column chunks
    nw = g // w               # chunks per row-tile

    xf = nc.dram_tensor("xf", [b, g], f32, kind="ExternalInput").ap()
    hnt = nc.dram_tensor("hnt", [D, b], f32, kind="ExternalInput").ap()
    latTs = nc.dram_tensor("latTs", [D, s], f32, kind="ExternalInput").ap()
    xs = nc.dram_tensor("xs", [s, g], f32, kind="ExternalInput").ap()
    unif = nc.dram_tensor("unif", [s, g], f32, kind="ExternalInput").ap()
    rnd = nc.dram_tensor("rnd", [tp, nt], i32, kind="ExternalInput").ap()
    out = nc.dram_tensor("out", [s, g], f32, kind="ExternalOutput").ap()

    with tile.TileContext(nc) as tc:
        with (
            tc.tile_pool(name="const", bufs=1) as cpool,
            tc.tile_pool(name="simp", bufs=1) as simpool,
            tc.tile_pool(name="gat", bufs=2) as gpool,
            tc.tile_pool(name="xch", bufs=2) as xpool,
            tc.tile_pool(name="uch", bufs=2) as upool,
            tc.tile_pool(name="msk", bufs=2) as mpool,
            tc.tile_pool(name="small", bufs=2) as spool,
            tc.tile_pool(name="mm", bufs=4, space="PSUM") as mmpool,
        ):
            iota8 = cpool.tile([tp, 8], i32)
            nc.gpsimd.iota(iota8[:], pattern=[[1, 8]], base=-1, channel_multiplier=0)
            rnd_sb = cpool.tile([tp, nt], i32)
            nc.sync.dma_start(rnd_sb[:], rnd)
            latTs_sb = cpool.tile([D, s], f32)
            nc.sync.dma_start(latTs_sb[:], latTs)
            hnT = cpool.tile([D, b], f32)
            nc.sync.dma_start(hnT[:], hnt)

            for t in range(nt):
                # x/unif chunk loads do not depend on the neighbor index:
                # emit them first so they fill the pre-gather window.
                xcs, mks = [], []
                for ci in range(nw):
                    c0 = ci * w
                    xc = xpool.tile([tp, w], f32, tag="xc")
                    nc.sync.dma_start(xc[:], xs[ds(t * tp, tp), ds(c0, w)])
                    uc = upool.tile([tp, w], f32, tag="uc")
                    nc.sync.dma_start(uc[:], unif[ds(t * tp, tp), ds(c0, w)])
                    mk = mpool.tile([tp, w], i8, tag="mk")
                    nc.vector.tensor_scalar(
                        out=mk[:], in0=uc[:], scalar1=MIX, scalar2=None, op0=OP.is_lt
                    )
                    xcs.append(xc)
                    mks.append(mk)

                sim = simpool.tile([tp, b], f32, tag="sim")
                for j in range(nj):
                    ps = mmpool.tile([tp, 512], f32, tag="mm")
                    nc.tensor.matmul(
                        ps[:],
                        lhsT=latTs_sb[:, ts(t, tp)],
                        rhs=hnT[:, ts(j, 512)],
                        start=True,
                        stop=True,
                    )
                    nc.scalar.copy(out=sim[:, ts(j, 512)], in_=ps[:])
                mx = spool.tile([tp, 8], f32, tag="mx")
                nc.vector.max(out=mx[:], in_=sim[:])
                mi = spool.tile([tp, 8], u32, tag="mi")
                nc.vector.max_index(out=mi[:], in_max=mx[:], in_values=sim[:])
                eq = spool.tile([tp, 8], i32, tag="eq")
                nc.vector.tensor_tensor(
                    out=eq[:],
                    in0=iota8[:],
                    in1=rnd_sb[:, t : t + 1].to_broadcast([tp, 8]),
                    op=OP.is_equal,
                )
                pr = spool.tile([tp, 8], i32, tag="pr")
                nc.vector.tensor_tensor(
                    out=pr[:], in0=eq[:], in1=mi[:].bitcast(i32), op=OP.mult
                )
                nbr = spool.tile([tp, 1], i32, tag="nbr")
                with nc.allow_low_precision(reason="int32 index select, exact"):
                    nc.vector.reduce_sum(out=nbr[:], in_=pr[:], axis=AX.X)

                for ci in range(nw):
                    gt = gpool.tile([tp, w], tag="gt", dtype=f32)
                    nc.gpsimd.indirect_dma_start(
                        out=gt[:],
                        out_offset=None,
                        in_=xf,
                        in_offset=bass.IndirectOffsetOnAxis(ap=nbr[:, :1], axis=0),
                        element_offset=ci * w,
                    )
                    nc.vector.copy_predicated(
                        out=gt[:], mask=mks[ci][:], data=xcs[ci][:]
                    )
                    nc.sync.dma_start(out[ds(t * tp, tp), ds(ci * w, w)], gt[:])
    return nc


def _get_prog():
    global _PROG
    if _PROG is None:
        from concourse import bacc

        nc = bacc.Bacc(
            "TRN2", target_bir_lowering=False, debug=False, num_devices=N_CORES
        )
        build_program(nc, B, G, S, W)
        nc.compile()
        _PROG = nc
    return _PROG


def make_hnt(latent):
    lat64 = latent.astype(np.float64)
    hn = lat64 / np.sqrt((lat64 * lat64).sum(axis=1, keepdims=True))
    return np.ascontiguousarray(hn.T.astype(np.float32))


def make_in_maps(x, latent, rand_idx, unif):
    x = np.ascontiguousarray(np.asarray(x, dtype=np.float32))
    latent = np.ascontiguousarray(np.asarray(latent, dtype=np.float32))
    rand_idx = np.asarray(rand_idx, dtype=np.int32)
    unif = np.ascontiguousarray(np.asarray(unif, dtype=np.float32))
    hnt = make_hnt(latent)
    in_maps = []
    for c in range(N_CORES):
        r0 = c * S
        in_maps.append(
            {
                "xf": x,
                "hnt": hnt,
                "latTs": np.ascontiguousarray(latent[r0 : r0 + S].T),
                "xs": x[r0 : r0 + S],
                "unif": unif[r0 : r0 + S],
                "rnd": np.ascontiguousarray(
                    rand_idx[r0 : r0 + S].reshape(NT, TP).T
                ),
            }
        )
    return in_maps


def kernel(x, latent, rand_idx, unif):
    from concourse.bass_utils import run_bass_kernel_spmd

    nc = _get_prog()
    in_maps = make_in_maps(x, latent, rand_idx, unif)
    res = run_bass_kernel_spmd(nc, in_maps, core_ids=list(range(N_CORES)))
    return np.concatenate([res.results[c]["out"] for c in range(N_CORES)], axis=0)
